# revision 25
# baseline (speedup 1.0000x reference)
"""ArrowLoRA MoE routing kernel for 8 TRN2 NeuronCores.

Math (per token t of 8192, F=2048, E=16 experts, R=16, O=2048):
    sim   = |x @ protos.T|                       (t, E)
    coeff = softmax(top4-masked sim)             (t, E)
    z     = x @ Acat.T                           (t, E*R)   Acat = A_stack.reshape(256, F)
    delta = scaling * (coeff-weighted z) @ Bcat  (t, O)     Bcat = B_stack.transpose(0,2,1).reshape(256, O)

Strategy: token-parallel across 8 cores (1024 tokens each), weights
replicated, no collectives. Host pre-transposes/casts x to fp16 (bf16 is
NOT enough precision for the top-4 routing: it flips picks for ~0.7% of
tokens and fails the 2e-2 gate; fp16 misroutes only ~0.05% and lands at
rel_err ~7e-3 including an fp16 output).

Per core: one fused matmul per 128-token tile produces z and sim together
(moving operand = [Acat.T | protos.T], 272 cols); top-4 via the DVE top-8
`max` op; softmax on batched [128, 8, 16] tiles; zw = z * coeff via a
broadcast AP; PE-transpose of zw to put E*R on partitions; second matmul
against Bcat (scaling folded in on host); fp16 output upcast on host.
"""

import os

import numpy as np

import concourse.bass as bass
import concourse.mybir as mybir
from concourse import bacc
from concourse.bass import ts
from concourse.bass_utils import run_bass_kernel_spmd
from concourse.tile import TileContext

# Problem shape (hardcoded per spec).
B, S, F, E, R, O = 4, 2048, 2048, 16, 16, 2048
TOPK = 4
NCORES = 8
T = B * S                  # 8192 tokens
TPC = T // NCORES          # 1024 tokens per core
NT = TPC // 128            # 8 token tiles per core
FCH = F // 128             # 16 F chunks (contraction)
ER = E * R                 # 256
ERCH = ER // 128           # 2 er chunks
WCOLS = ER + E             # 272: [z cols | sim cols]

F16 = mybir.dt.float16
F32 = mybir.dt.float32

_CACHE = {}

LAST_RESULTS = None  # BassKernelResults of the most recent run (for test.py)


def build_nc():
    # Bacc (not raw Bass): its compile passes legalize sync waits to the
    # 1-wait-per-instruction HW limit (move_matmul_waits_to_ldweights +
    # generate_event_semaphores) — raw Bass graphs with recycled PSUM slots
    # fail walrus codegen with "Too many sync wait commands".
    nc = bacc.Bacc(target_bir_lowering=False)

    xT = nc.declare_dram_parameter("xT", [FCH // 4, 128, 4, TPC], F16, isOutput=False)
    W = nc.declare_dram_parameter("W", [128, FCH, WCOLS], F16, isOutput=False)
    Bc = nc.declare_dram_parameter("Bc", [ERCH, 128, O], F16, isOutput=False)
    ident = nc.declare_dram_parameter("ident", [128, 128], F16, isOutput=False)
    # out pairs two 128-token tiles per partition row -> 8KB DMA runs
    out = nc.declare_dram_parameter("out", [NT // 2, 128, 2, O], F16, isOutput=True)

    XG = 4                     # x chunks per DMA group (8KB runs/partition)
    NG = FCH // XG

    with TileContext(nc) as tc:
        with (
            tc.tile_pool(name="weights", bufs=1) as wpool,
            tc.tile_pool(name="xdata", bufs=1) as xpool,
            tc.tile_pool(name="small", bufs=1) as small,
            tc.tile_pool(name="work", bufs=3) as work,
            tc.tile_pool(name="psum_zs", bufs=2, space="PSUM") as psum_zs,
            tc.tile_pool(name="psum_t", bufs=2, space="PSUM") as psum_t,
            tc.tile_pool(name="psum_d", bufs=4, space="PSUM") as psum_d,
        ):
            # ---- input DMAs: few big ones (>=8KB contiguous runs), issue
            # spread across idle engine sequencers so the ~600ns DIRECT2D
            # issue costs run in parallel; W and x group 0 first ----
            w_sb = wpool.tile([128, FCH, WCOLS], F16, name="w_sb")
            nc.sync.dma_start(out=w_sb, in_=W[:, :, :])
            x_sb = [
                xpool.tile([128, XG, TPC], F16, name=f"x_sb{g}", tag=f"x{g}")
                for g in range(NG)
            ]
            nc.gpsimd.dma_start(out=x_sb[0], in_=xT[0])
            nc.sync.dma_start(out=x_sb[1], in_=xT[1])
            nc.gpsimd.dma_start(out=x_sb[2], in_=xT[2])
            nc.sync.dma_start(out=x_sb[3], in_=xT[3])
            id_sb = wpool.tile([128, 128], F16, name="id_sb")
            nc.gpsimd.dma_start(out=id_sb, in_=ident[:, :])
            bc_sb = wpool.tile([128, ERCH, O], F16, name="bc_sb")
            nc.gpsimd.dma_start(out=bc_sb, in_=Bc[:, :, :].rearrange("c p o -> p c o"))

            # ---- routing stat tiles (written per token tile) ----
            sim_all = small.tile([128, NT, E], F32, name="sim_all")
            m8_all = small.tile([128, NT, 8], F32, name="m8_all")
            m1n_all = small.tile([128, NT], F32, name="m1n_all")
            e_all = small.tile([128, NT, E], F32, name="e_all")
            ge_all = small.tile([128, NT, E], F32, name="ge_all")
            em_all = small.tile([128, NT, E], F32, name="em_all")
            den_all = small.tile([128, NT], F32, name="den_all")
            rcp_all = small.tile([128, NT], F32, name="rcp_all")
            coeff_all = small.tile([128, NT, E], F32, name="coeff_all")

            for i in range(NT):
                # ---- fused z+sim matmul (accumulate over F chunks) ----
                zs = psum_zs.tile([128, WCOLS], F32, name=f"zs{i}", tag="zs")
                for c in range(FCH):
                    nc.tensor.matmul(
                        zs,
                        lhsT=x_sb[c // XG][:, c % XG, ts(i, 128)],
                        rhs=w_sb[:, c, :],
                        start=(c == 0),
                        stop=(c == FCH - 1),
                    )
                # ---- top-4 softmax routing for this tile ----
                sim = sim_all[:, i, :]
                nc.scalar.activation(
                    sim, zs[:, ER:WCOLS], mybir.ActivationFunctionType.Abs
                )
                nc.vector.max(m8_all[:, i, :], sim)
                nc.vector.tensor_scalar_mul(
                    m1n_all[:, i:i + 1], m8_all[:, i, 0:1], -1.0
                )
                nc.scalar.activation(
                    e_all[:, i, :], sim, mybir.ActivationFunctionType.Exp,
                    bias=m1n_all[:, i:i + 1],
                )
                nc.vector.tensor_scalar(
                    ge_all[:, i, :], sim, m8_all[:, i, 3:4], None,
                    op0=mybir.AluOpType.is_ge,
                )
                nc.vector.tensor_tensor(
                    em_all[:, i, :], e_all[:, i, :], ge_all[:, i, :],
                    mybir.AluOpType.mult,
                )
                nc.vector.tensor_reduce(
                    den_all[:, i:i + 1], em_all[:, i, :],
                    axis=mybir.AxisListType.X, op=mybir.AluOpType.add,
                )
                nc.vector.reciprocal(rcp_all[:, i:i + 1], den_all[:, i:i + 1])
                nc.vector.tensor_scalar(
                    coeff_all[:, i, :], em_all[:, i, :], rcp_all[:, i:i + 1],
                    None, op0=mybir.AluOpType.mult,
                )
                # ---- weight z by coeff straight out of PSUM ----
                zw = work.tile([128, E, R], F16, name=f"zw{i}", tag="zw")
                nc.vector.tensor_tensor(
                    zw,
                    zs[:, 0:ER].rearrange("p (e r) -> p e r", r=R),
                    coeff_all[:, i, :, None].to_broadcast([128, E, R]),
                    mybir.AluOpType.mult,
                )
                # ---- transpose zw to put E*R on partitions ----
                zwT = work.tile([128, ERCH, 128], F16, name=f"zwT{i}", tag="zwT")
                for h in range(ERCH):
                    tp = psum_t.tile([128, 128], F16, name=f"tp{i}_{h}", tag="tp")
                    nc.tensor.transpose(
                        tp, zw.rearrange("p e r -> p (e r)")[:, ts(h, 128)], id_sb
                    )
                    nc.scalar.activation(
                        zwT[:, h, :], tp, mybir.ActivationFunctionType.Copy
                    )
                # ---- second matmul against Bcat + store (2 tiles/DMA) ----
                if i % 2 == 0:
                    out_sb = work.tile([128, 2, O], F16, name=f"out_sb{i // 2}",
                                       tag="out_sb", bufs=2)
                for q in range(4):
                    dq = psum_d.tile([128, 512], F32, name=f"d{i}_{q}", tag="d")
                    for ch in range(ERCH):
                        nc.tensor.matmul(
                            dq,
                            lhsT=zwT[:, ch, :],
                            rhs=bc_sb[:, ch, ts(q, 512)],
                            start=(ch == 0),
                            stop=(ch == ERCH - 1),
                        )
                    if q % 2 == 0:
                        nc.scalar.activation(
                            out_sb[:, i % 2, ts(q, 512)], dq,
                            mybir.ActivationFunctionType.Copy,
                        )
                    else:
                        nc.vector.tensor_copy(out=out_sb[:, i % 2, ts(q, 512)], in_=dq)
                if i % 2 == 1:
                    nc.gpsimd.dma_start(out=out[i // 2], in_=out_sb)

    nc.finalize()  # runs Bacc.compile(): reg alloc + sync-wait legalization
    return nc


def _host_prep(x, prototypes, A_stack, B_stack, scaling):
    tok = np.ascontiguousarray(x.reshape(T, F))

    Acat = A_stack.reshape(ER, F)
    Wh = np.concatenate([Acat.T, prototypes.T], axis=1)        # (F, 272)
    # W dram layout [128, FCH, WCOLS]: partition-major so the single DMA
    # reads 8.7KB contiguous per partition.
    Wh = np.ascontiguousarray(
        Wh.reshape(FCH, 128, WCOLS).transpose(1, 0, 2)
    ).astype(np.float16)

    Bcat = (B_stack.transpose(0, 2, 1).reshape(ER, O) * float(scaling))
    Bch = Bcat.reshape(ERCH, 128, O).astype(np.float16)

    identh = np.eye(128, dtype=np.float16)

    in_maps = []
    for core in range(NCORES):
        shard = tok[core * TPC:(core + 1) * TPC]               # (TPC, F)
        # xT dram layout [FCH//4, 128, 4, TPC]: groups of 4 F-chunks,
        # partition-major within a group -> 8KB contiguous per partition.
        xTh = (
            shard.T.reshape(FCH // 4, 4, 128, TPC)
            .transpose(0, 2, 1, 3)
            .astype(np.float16)
        )
        in_maps.append({
            "xT": np.ascontiguousarray(xTh),
            "W": Wh,
            "Bc": Bch,
            "ident": identh,
        })
    return in_maps


def _setup_axon_tracing():
    """Make trace=True work in this container: register the NTFF profile
    hook that the image's antenv lacks, and neuter upload_artifacts (no
    artifact store here). Best-effort — failures just disable tracing."""
    import sys
    import types

    import concourse.bass_utils as bu

    bu.upload_artifacts = lambda tmpdir: "local://" + tmpdir
    try:
        from antenv.axon_hooks import get_axon_ntff_profile_hook  # noqa: F401
        return
    except ImportError:
        pass
    import antenv
    from trn_agent_boot.trn_boot import _ntff_profile_via_ctypes

    mod = types.ModuleType("antenv.axon_hooks")
    state = {"hook": _ntff_profile_via_ctypes("/opt/axon/libaxon_pjrt.so")}
    mod.set_axon_ntff_profile_hook = lambda h: state.__setitem__("hook", h)
    mod.get_axon_ntff_profile_hook = lambda: state["hook"]
    antenv.axon_hooks = mod
    sys.modules["antenv.axon_hooks"] = mod


def kernel(x, prototypes, A_stack, B_stack, scaling, top_k):
    global LAST_RESULTS
    assert int(top_k) == TOPK, f"kernel hardcodes top_k={TOPK}, got {top_k}"
    assert x.shape == (B, S, F)

    if "nc" not in _CACHE:
        _CACHE["nc"] = build_nc()
    nc = _CACHE["nc"]

    in_maps = _host_prep(
        np.asarray(x, dtype=np.float32),
        np.asarray(prototypes, dtype=np.float32),
        np.asarray(A_stack, dtype=np.float32),
        np.asarray(B_stack, dtype=np.float32),
        np.asarray(scaling, dtype=np.float32),
    )

    trace = os.environ.get("KERNEL_TRACE", "0") == "1"
    if trace:
        try:
            _setup_axon_tracing()
        except Exception as e:  # tracing is optional; never fail the run
            print(f"tracing setup failed ({e}); running without trace")
            trace = False
    res = run_bass_kernel_spmd(nc, in_maps, core_ids=list(range(NCORES)), trace=trace)
    LAST_RESULTS = res

    outs = [
        res.results[i]["out"].transpose(0, 2, 1, 3).reshape(TPC, O)
        for i in range(NCORES)
    ]
    full = np.concatenate(outs, axis=0).astype(np.float32)
    return full.reshape(B, S, O)


# revision 27
# speedup vs baseline: 1.3501x; 1.3501x over previous
"""ArrowLoRA MoE routing kernel for 8 TRN2 NeuronCores.

Math (per token t of 8192, F=2048, E=16 experts, R=16, O=2048):
    sim   = |x @ protos.T|                       (t, E)
    coeff = softmax(top4-masked sim)             (t, E)
    z     = x @ Acat.T                           (t, E*R)   Acat = A_stack.reshape(256, F)
    delta = scaling * (coeff-weighted z) @ Bcat  (t, O)     Bcat = B_stack.transpose(0,2,1).reshape(256, O)

Strategy: token-parallel across 8 cores (1024 tokens each), weights
replicated, no collectives. Host pre-transposes/casts x to fp16 (bf16 is
NOT enough precision for the top-4 routing: it flips picks for ~0.7% of
tokens and fails the 2e-2 gate; fp16 misroutes only ~0.05% and lands at
rel_err ~7e-3 including an fp16 output).

Per core: one fused matmul per 128-token tile produces z and sim together
(moving operand = [Acat.T | protos.T], 272 cols); top-4 via the DVE top-8
`max` op; softmax on batched [128, 8, 16] tiles; zw = z * coeff via a
broadcast AP; PE-transpose of zw to put E*R on partitions; second matmul
against Bcat (scaling folded in on host); fp16 output upcast on host.
"""

import os

import numpy as np

import concourse.bass as bass
import concourse.mybir as mybir
from concourse import bacc
from concourse.bass import ts
from concourse.bass_utils import run_bass_kernel_spmd
from concourse.tile import TileContext

# Problem shape (hardcoded per spec).
B, S, F, E, R, O = 4, 2048, 2048, 16, 16, 2048
TOPK = 4
NCORES = 8
T = B * S                  # 8192 tokens
TPC = T // NCORES          # 1024 tokens per core
NT = TPC // 128            # 8 token tiles per core
FCH = F // 128             # 16 F chunks (contraction)
ER = E * R                 # 256
ERCH = ER // 128           # 2 er chunks
WCOLS = ER + E             # 272: [z cols | sim cols]

F16 = mybir.dt.float16
F32 = mybir.dt.float32

_CACHE = {}

LAST_RESULTS = None  # BassKernelResults of the most recent run (for test.py)


def build_nc():
    # Bacc (not raw Bass): its compile passes legalize sync waits to the
    # 1-wait-per-instruction HW limit (move_matmul_waits_to_ldweights +
    # generate_event_semaphores) — raw Bass graphs with recycled PSUM slots
    # fail walrus codegen with "Too many sync wait commands".
    nc = bacc.Bacc(target_bir_lowering=False)

    xT = nc.declare_dram_parameter("xT", [FCH // 4, 128, 4, TPC], F16, isOutput=False)
    W = nc.declare_dram_parameter("W", [128, FCH, WCOLS], F16, isOutput=False)
    Bc = nc.declare_dram_parameter("Bc", [ERCH, 128, O], F16, isOutput=False)
    ident = nc.declare_dram_parameter("ident", [128, 128], F16, isOutput=False)
    # out pairs two 128-token tiles per partition row -> 8KB DMA runs
    out = nc.declare_dram_parameter("out", [NT // 2, 128, 2, O], F16, isOutput=True)

    XG = 4                     # x chunks per DMA group (8KB runs/partition)
    NG = FCH // XG

    with TileContext(nc) as tc:
        with (
            tc.tile_pool(name="weights", bufs=1) as wpool,
            tc.tile_pool(name="xdata", bufs=1) as xpool,
            tc.tile_pool(name="small", bufs=1) as small,
            tc.tile_pool(name="work", bufs=3) as work,
            tc.tile_pool(name="psum_zs", bufs=2, space="PSUM") as psum_zs,
            tc.tile_pool(name="psum_t", bufs=2, space="PSUM") as psum_t,
            tc.tile_pool(name="psum_d", bufs=4, space="PSUM") as psum_d,
        ):
            # ---- input DMAs: few big ones (>=8KB contiguous runs), issue
            # spread across idle engine sequencers so the ~600ns DIRECT2D
            # issue costs run in parallel; W and x group 0 first ----
            x_sb = [
                xpool.tile([128, XG, TPC], F16, name=f"x_sb{g}", tag=f"x{g}")
                for g in range(NG)
            ]
            w_sb = wpool.tile([128, FCH, WCOLS], F16, name="w_sb")
            nc.sync.dma_start(out=x_sb[0], in_=xT[0])
            nc.gpsimd.dma_start(out=w_sb, in_=W[:, :, :])
            nc.sync.dma_start(out=x_sb[1], in_=xT[1])
            nc.gpsimd.dma_start(out=x_sb[2], in_=xT[2])
            nc.sync.dma_start(out=x_sb[3], in_=xT[3])
            id_sb = wpool.tile([128, 128], F16, name="id_sb")
            nc.gpsimd.dma_start(out=id_sb, in_=ident[:, :])
            bc_sb = wpool.tile([128, ERCH, O], F16, name="bc_sb")
            nc.gpsimd.dma_start(out=bc_sb, in_=Bc[:, :, :].rearrange("c p o -> p c o"))

            # ---- routing stat tiles (written per token tile) ----
            sim_all = small.tile([128, NT, E], F32, name="sim_all")
            m8_all = small.tile([128, NT, 8], F32, name="m8_all")
            e_all = small.tile([128, NT, E], F32, name="e_all")
            ge_all = small.tile([128, NT, E], F32, name="ge_all")
            em_all = small.tile([128, NT, E], F32, name="em_all")
            den_all = small.tile([128, NT], F32, name="den_all")
            rcp_all = small.tile([128, NT], F32, name="rcp_all")
            coeff_all = small.tile([128, NT, E], F32, name="coeff_all")

            zs_tiles = [None] * NT
            out_sb_box = [None]

            def emit_mm1(i):
                zs = psum_zs.tile([128, WCOLS], F32, name=f"zs{i}", tag="zs")
                zs_tiles[i] = zs
                for c in range(FCH):
                    nc.tensor.matmul(
                        zs,
                        lhsT=x_sb[c // XG][:, c % XG, ts(i, 128)],
                        rhs=w_sb[:, c, :],
                        start=(c == 0),
                        stop=(c == FCH - 1),
                    )

            def emit_tail(i):
                zs = zs_tiles[i]
                # top-4 softmax routing: |sim| then exp back-to-back on ACT
                # (no max-shift needed: |sim| <= ~6 so exp <= ~400, safe in
                # f32 and fp16 range); top-8 runs on DVE in parallel.
                sim = sim_all[:, i, :]
                nc.scalar.activation(
                    sim, zs[:, ER:WCOLS], mybir.ActivationFunctionType.Abs
                )
                nc.scalar.activation(
                    e_all[:, i, :], sim, mybir.ActivationFunctionType.Exp
                )
                nc.vector.max(m8_all[:, i, :], sim)
                nc.vector.tensor_scalar(
                    ge_all[:, i, :], sim, m8_all[:, i, 3:4], None,
                    op0=mybir.AluOpType.is_ge,
                )
                nc.vector.tensor_tensor(
                    em_all[:, i, :], e_all[:, i, :], ge_all[:, i, :],
                    mybir.AluOpType.mult,
                )
                nc.vector.tensor_reduce(
                    den_all[:, i:i + 1], em_all[:, i, :],
                    axis=mybir.AxisListType.X, op=mybir.AluOpType.add,
                )
                nc.vector.reciprocal(rcp_all[:, i:i + 1], den_all[:, i:i + 1])
                nc.vector.tensor_scalar(
                    coeff_all[:, i, :], em_all[:, i, :], rcp_all[:, i:i + 1],
                    None, op0=mybir.AluOpType.mult,
                )
                # weight z by coeff straight out of PSUM
                zw = work.tile([128, E, R], F16, name=f"zw{i}", tag="zw")
                nc.vector.tensor_tensor(
                    zw,
                    zs[:, 0:ER].rearrange("p (e r) -> p e r", r=R),
                    coeff_all[:, i, :, None].to_broadcast([128, E, R]),
                    mybir.AluOpType.mult,
                )
                # transpose zw to put E*R on partitions
                zwT = work.tile([128, ERCH, 128], F16, name=f"zwT{i}", tag="zwT")
                for h in range(ERCH):
                    tp = psum_t.tile([128, 128], F16, name=f"tp{i}_{h}", tag="tp")
                    nc.tensor.transpose(
                        tp, zw.rearrange("p e r -> p (e r)")[:, ts(h, 128)], id_sb
                    )
                    nc.scalar.activation(
                        zwT[:, h, :], tp, mybir.ActivationFunctionType.Copy
                    )
                # second matmul against Bcat + store (2 tiles per DMA)
                if i % 2 == 0:
                    out_sb_box[0] = work.tile(
                        [128, 2, O], F16, name=f"out_sb{i // 2}",
                        tag="out_sb", bufs=2,
                    )
                out_sb = out_sb_box[0]
                for q in range(4):
                    dq = psum_d.tile([128, 512], F32, name=f"d{i}_{q}", tag="d")
                    for ch in range(ERCH):
                        nc.tensor.matmul(
                            dq,
                            lhsT=zwT[:, ch, :],
                            rhs=bc_sb[:, ch, ts(q, 512)],
                            start=(ch == 0),
                            stop=(ch == ERCH - 1),
                        )
                    if q % 2 == 0:
                        nc.scalar.activation(
                            out_sb[:, i % 2, ts(q, 512)], dq,
                            mybir.ActivationFunctionType.Copy,
                        )
                    else:
                        nc.vector.tensor_copy(out=out_sb[:, i % 2, ts(q, 512)], in_=dq)
                if i % 2 == 1:
                    nc.gpsimd.dma_start(out=out[i // 2], in_=out_sb)

            # software-pipelined emission: mm1 of tile i+1 is issued before
            # tile i's routing/second-matmul so PE never waits on the
            # DVE/ACT routing chain.
            emit_mm1(0)
            for i in range(NT):
                if i + 1 < NT:
                    emit_mm1(i + 1)
                emit_tail(i)

    nc.finalize()  # runs Bacc.compile(): reg alloc + sync-wait legalization
    return nc


def _host_prep(x, prototypes, A_stack, B_stack, scaling):
    tok = np.ascontiguousarray(x.reshape(T, F))

    Acat = A_stack.reshape(ER, F)
    Wh = np.concatenate([Acat.T, prototypes.T], axis=1)        # (F, 272)
    # W dram layout [128, FCH, WCOLS]: partition-major so the single DMA
    # reads 8.7KB contiguous per partition.
    Wh = np.ascontiguousarray(
        Wh.reshape(FCH, 128, WCOLS).transpose(1, 0, 2)
    ).astype(np.float16)

    Bcat = (B_stack.transpose(0, 2, 1).reshape(ER, O) * float(scaling))
    Bch = Bcat.reshape(ERCH, 128, O).astype(np.float16)

    identh = np.eye(128, dtype=np.float16)

    in_maps = []
    for core in range(NCORES):
        shard = tok[core * TPC:(core + 1) * TPC]               # (TPC, F)
        # xT dram layout [FCH//4, 128, 4, TPC]: groups of 4 F-chunks,
        # partition-major within a group -> 8KB contiguous per partition.
        xTh = (
            shard.T.reshape(FCH // 4, 4, 128, TPC)
            .transpose(0, 2, 1, 3)
            .astype(np.float16)
        )
        in_maps.append({
            "xT": np.ascontiguousarray(xTh),
            "W": Wh,
            "Bc": Bch,
            "ident": identh,
        })
    return in_maps


def _setup_axon_tracing():
    """Make trace=True work in this container: register the NTFF profile
    hook that the image's antenv lacks, and neuter upload_artifacts (no
    artifact store here). Best-effort — failures just disable tracing."""
    import sys
    import types

    import concourse.bass_utils as bu

    bu.upload_artifacts = lambda tmpdir: "local://" + tmpdir
    try:
        from antenv.axon_hooks import get_axon_ntff_profile_hook  # noqa: F401
        return
    except ImportError:
        pass
    import antenv
    from trn_agent_boot.trn_boot import _ntff_profile_via_ctypes

    mod = types.ModuleType("antenv.axon_hooks")
    state = {"hook": _ntff_profile_via_ctypes("/opt/axon/libaxon_pjrt.so")}
    mod.set_axon_ntff_profile_hook = lambda h: state.__setitem__("hook", h)
    mod.get_axon_ntff_profile_hook = lambda: state["hook"]
    antenv.axon_hooks = mod
    sys.modules["antenv.axon_hooks"] = mod


def kernel(x, prototypes, A_stack, B_stack, scaling, top_k):
    global LAST_RESULTS
    assert int(top_k) == TOPK, f"kernel hardcodes top_k={TOPK}, got {top_k}"
    assert x.shape == (B, S, F)

    if "nc" not in _CACHE:
        _CACHE["nc"] = build_nc()
    nc = _CACHE["nc"]

    in_maps = _host_prep(
        np.asarray(x, dtype=np.float32),
        np.asarray(prototypes, dtype=np.float32),
        np.asarray(A_stack, dtype=np.float32),
        np.asarray(B_stack, dtype=np.float32),
        np.asarray(scaling, dtype=np.float32),
    )

    trace = os.environ.get("KERNEL_TRACE", "0") == "1"
    if trace:
        try:
            _setup_axon_tracing()
        except Exception as e:  # tracing is optional; never fail the run
            print(f"tracing setup failed ({e}); running without trace")
            trace = False
    res = run_bass_kernel_spmd(nc, in_maps, core_ids=list(range(NCORES)), trace=trace)
    LAST_RESULTS = res

    outs = [
        res.results[i]["out"].transpose(0, 2, 1, 3).reshape(TPC, O)
        for i in range(NCORES)
    ]
    full = np.concatenate(outs, axis=0).astype(np.float32)
    return full.reshape(B, S, O)


# revision 28
# speedup vs baseline: 1.3946x; 1.0330x over previous
"""ArrowLoRA MoE routing kernel for 8 TRN2 NeuronCores.

Math (per token t of 8192, F=2048, E=16 experts, R=16, O=2048):
    sim   = |x @ protos.T|; coeff = softmax(top4-masked sim)
    z     = x @ Acat.T;     delta = scaling * (coeff * z per expert) @ Bcat

Strategy: token-parallel across 8 cores (1024 tokens each), weights
replicated, no collectives. Host pre-transposes/casts x to fp16 (bf16 is
NOT enough precision for top-4 routing: it flips picks for ~0.7% of
tokens and fails the 2e-2 gate; fp16 misroutes only ~0.05%, landing at
rel_err ~7.4e-3 with an fp16 output). Raw Bass engine streams with
hand-placed semaphores (a Tile version measured ~73us; this is ~64us —
less start/end overhead, minimal PE waits, software-pipelined depth so
PE never stalls on the DVE/ACT routing chain).

Pipeline per core (1024 tokens = 8 tiles of 128):
  PE : mm1(i) [16 accum matmuls -> zs psum]  |  transp(i-1), mm2(i-1)
  ACT: |sim| -> exp (no max-shift; |sim|<=~6) ; zwT copies ; half epilogue
  DVE: top8 -> mask -> softmax -> coeff ; zw = z*coeff (reads PSUM) ; half epilogue
  SP : input DMAs (x groups)   GpSimd: W/Bc/ident DMAs + output DMAs
"""

import os

import numpy as np

import concourse.bass as bass
import concourse.mybir as mybir
from concourse import bacc
from concourse.bass import ts
from concourse.bass_utils import run_bass_kernel_spmd

B, S, F, E, R, O = 4, 2048, 2048, 16, 16, 2048
TOPK = 4
NCORES = 8
T = B * S
TPC = T // NCORES          # 1024
NT = TPC // 128            # 8
FCH = F // 128             # 16
ER = E * R                 # 256
ERCH = ER // 128           # 2
WCOLS = ER + E             # 272
XG = 4
NG = FCH // XG

F16 = mybir.dt.float16
F32 = mybir.dt.float32
ALU = mybir.AluOpType
AF = mybir.ActivationFunctionType

_CACHE = {}
LAST_RESULTS = None


def build_nc():
    nc = bacc.Bacc(target_bir_lowering=False)

    xT = nc.declare_dram_parameter("xT", [NG, 128, XG, TPC], F16, isOutput=False)
    W = nc.declare_dram_parameter("W", [128, FCH, WCOLS], F16, isOutput=False)
    Bc = nc.declare_dram_parameter("Bc", [ERCH, 128, O], F16, isOutput=False)
    ident = nc.declare_dram_parameter("ident", [128, 128], F16, isOutput=False)
    out = nc.declare_dram_parameter("out", [NT // 2, 128, 2, O], F16, isOutput=True)

    # ---- SBUF ----
    x_sb = nc.alloc_sbuf_tensor("x_sb", [128, FCH, TPC], F16).ap()
    w_sb = nc.alloc_sbuf_tensor("w_sb", [128, FCH, WCOLS], F16).ap()
    bc_sb = nc.alloc_sbuf_tensor("bc_sb", [128, ERCH, O], F16).ap()
    id_sb = nc.alloc_sbuf_tensor("id_sb", [128, 128], F16).ap()
    sim_a = nc.alloc_sbuf_tensor("sim_a", [128, NT, E], F32).ap()
    m8_a = nc.alloc_sbuf_tensor("m8_a", [128, NT, 8], F32).ap()
    e_a = nc.alloc_sbuf_tensor("e_a", [128, NT, E], F32).ap()
    ge_a = nc.alloc_sbuf_tensor("ge_a", [128, NT, E], F32).ap()
    em_a = nc.alloc_sbuf_tensor("em_a", [128, NT, E], F32).ap()
    den_a = nc.alloc_sbuf_tensor("den_a", [128, NT], F32).ap()
    rcp_a = nc.alloc_sbuf_tensor("rcp_a", [128, NT], F32).ap()
    co_a = nc.alloc_sbuf_tensor("co_a", [128, NT, E], F32).ap()
    zw_sb = nc.alloc_sbuf_tensor("zw_sb", [128, 2, E, R], F16).ap()      # 2 bufs
    zwT_sb = nc.alloc_sbuf_tensor("zwT_sb", [128, 2, ERCH, 128], F16).ap()
    out_sb = nc.alloc_sbuf_tensor("out_sb", [128, 2, 2, O], F16).ap()    # 2 pairs

    # ---- PSUM: 8 banks = zs x2 + tp x2 + delta x4 ----
    zs_ps = [nc.alloc_psum_tensor(f"zs{s}", [128, WCOLS], F32).ap() for s in range(2)]
    tp_ps = [nc.alloc_psum_tensor(f"tp{s}", [128, 128], F16).ap() for s in range(2)]
    d_ps = [nc.alloc_psum_tensor(f"d{q}", [128, 512], F32).ap() for q in range(4)]

    with (
        nc.Block(no_gpsimd_drain=True) as block,
        nc.semaphore("s_w") as s_w,
        nc.semaphore("s_wr") as s_wr,
        nc.semaphore("s_x0") as s_x0,
        nc.semaphore("s_x1") as s_x1,
        nc.semaphore("s_x2") as s_x2,
        nc.semaphore("s_x3") as s_x3,
        nc.semaphore("s_id") as s_id,
        nc.semaphore("s_bc") as s_bc,
        nc.semaphore("s_mm1") as s_mm1,
        nc.semaphore("s_abs") as s_abs,
        nc.semaphore("s_exp") as s_exp,
        nc.semaphore("s_zw") as s_zw,
        nc.semaphore("s_tp") as s_tp,
        nc.semaphore("s_zwt") as s_zwt,
        nc.semaphore("s_d") as s_d,
        nc.semaphore("s_epA") as s_epA,
        nc.semaphore("s_epV") as s_epV,
        nc.semaphore("s_out0") as s_out0,
        nc.semaphore("s_out1") as s_out1,
    ):
        s_x = [s_x0, s_x1, s_x2, s_x3]

        @block.sync
        def _(sp):
            # W chunk 0 first (70KB) so the very first matmul can start as
            # soon as x group 0 lands; the rest of W follows.
            sp.dma_start(out=w_sb[:, 0:1, :], in_=W[:, 0:1, :]).then_inc(s_w, 16)
            sp.dma_start(out=w_sb[:, 1:FCH, :], in_=W[:, 1:FCH, :]).then_inc(s_wr, 16)
            for g in range(NG):
                sp.dma_start(
                    out=x_sb[:, ts(g, XG), :], in_=xT[g]
                ).then_inc(s_x[g], 16)

        @block.gpsimd
        def _(gp):
            # gpsimd starts ~6us late (library load) - only late-needed DMAs.
            # Wait for x0 so bc/ident descriptors queue behind the
            # critical-path x groups, not ahead of them.
            gp.wait_ge(s_x0, 16)
            gp.dma_start(
                out=bc_sb, in_=Bc[:, :, :].rearrange("c p o -> p c o")
            ).then_inc(s_bc, 16)
            gp.dma_start(out=id_sb, in_=ident[:, :]).then_inc(s_id, 16)
            s_out = [s_out0, s_out1]
            for k in range(NT // 2 - 1):
                gp.wait_ge(s_epA, 4 * (k + 1))
                gp.wait_ge(s_epV, 4 * (k + 1))
                gp.dma_start(out=out[k], in_=out_sb[:, k % 2]).then_inc(s_out[k % 2], 16)
            kl = NT // 2 - 1
            for half in range(2):
                gp.wait_ge(s_epA, 4 * kl + 2 * (half + 1))
                gp.wait_ge(s_epV, 4 * kl + 2 * (half + 1))
                gp.dma_start(
                    out=out[kl][:, half:half + 1, :],
                    in_=out_sb[:, kl % 2, half:half + 1, :],
                ).then_inc(s_out[kl % 2], 16)
            gp.wait_ge(s_out0, 32)
            gp.wait_ge(s_out1, 48)

        @block.tensor
        def _(te):
            def pe_transp(j):
                te.wait_ge(s_zw, j + 1)
                if j == 0:
                    te.wait_ge(s_id, 16)
                for h in range(ERCH):
                    m = ERCH * j + h
                    # tp slot WAR vs ACT copy(m-2) is covered transitively:
                    # pe_mm2(j-1) already waited s_zwt >= 2j >= m-1.
                    te.matmul(
                        tp_ps[m % 2],
                        lhsT=zw_sb[:, j % 2].rearrange("p e r -> p (e r)")[:, ts(h, 128)],
                        rhs=id_sb,
                        is_transpose=True,
                    ).then_inc(s_tp, 1)

            def pe_mm2(j):
                if j == 0:
                    te.wait_ge(s_bc, 16)
                te.wait_ge(s_zwt, ERCH * j + 2)
                if j >= 1:                          # delta slot WAR: all of tile
                    te.wait_ge(s_epA, 2 * j)        # j-1's epilogues done
                    te.wait_ge(s_epV, 2 * j)
                for q in range(4):
                    for ch in range(ERCH):
                        mm = te.matmul(
                            d_ps[q],
                            lhsT=zwT_sb[:, j % 2, ch, :],
                            rhs=bc_sb[:, ch, ts(q, 512)],
                            start=(ch == 0),
                            stop=(ch == ERCH - 1),
                        )
                    mm.then_inc(s_d, 1)

            def emit_mm1(i):
                for c in range(FCH):
                    if i == 0 and c % XG == 0:
                        if c == 0:
                            te.wait_ge(s_w, 16)
                        te.wait_ge(s_x[c // XG], 16)
                    if i == 0 and c == 1:
                        te.wait_ge(s_wr, 16)
                    # zs slot WAR vs abs/zw(i-3) is transitive: pe_transp(i-3)
                    # (earlier in PE order) waited s_zw >= i-2, and zw(i-3)
                    # implies abs(i-3) through DVE program order.
                    mm = te.matmul(
                        zs_ps[i % 2],
                        lhsT=x_sb[:, c, ts(i, 128)],
                        rhs=w_sb[:, c, :],
                        start=(c == 0),
                        stop=(c == FCH - 1),
                    )
                mm.then_inc(s_mm1, 1)

            emit_mm1(0)
            emit_mm1(1)
            for i in range(NT):
                if i >= 1:
                    pe_mm2(i - 1)
                pe_transp(i)
                if i + 2 < NT:
                    emit_mm1(i + 2)
            pe_mm2(NT - 1)

        @block.scalar
        def _(act):
            def act_tail(j):
                for h in range(ERCH):
                    m = ERCH * j + h
                    act.wait_ge(s_tp, m + 1)
                    # zwT slot WAR vs mm2(j-2) covered: act_tail(j-1)'s ep
                    # waits reached s_d >= 4(j-1)+3 > 4(j-2)+4.
                    act.activation(
                        zwT_sb[:, j % 2, h, :], tp_ps[m % 2], AF.Copy
                    ).then_inc(s_zwt, 1)
                k, half = j // 2, j % 2
                for q in (0, 2):
                    act.wait_ge(s_d, 4 * j + q + 1)
                    if half == 0 and q == 0 and k >= 2:   # out_sb pair WAR
                        act.wait_ge([s_out0, s_out1][k % 2], 16 * (k // 2))
                    act.activation(
                        out_sb[:, k % 2, half, ts(q, 512)], d_ps[q],
                        AF.Copy,
                    ).then_inc(s_epA, 1)

            for i in range(NT):
                act.wait_ge(s_mm1, i + 1)
                act.activation(
                    sim_a[:, i, :], zs_ps[i % 2][:, ER:WCOLS], AF.Abs
                ).then_inc(s_abs, 1)
                act.drain()
                act.activation(
                    e_a[:, i, :], sim_a[:, i, :], AF.Exp
                ).then_inc(s_exp, 1)
                if i >= 1:
                    act_tail(i - 1)
            act_tail(NT - 1)

        @block.vector
        def _(ve):
            def dve_tail(j):
                k, half = j // 2, j % 2
                for q in (1, 3):
                    ve.wait_ge(s_d, 4 * j + q + 1)
                    if half == 0 and q == 1 and k >= 2:
                        ve.wait_ge([s_out0, s_out1][k % 2], 16 * (k // 2))
                    ve.tensor_copy(
                        out=out_sb[:, k % 2, half, ts(q, 512)], in_=d_ps[q]
                    ).then_inc(s_epV, 1)

            for i in range(NT):
                ve.wait_ge(s_abs, i + 1)
                ve.max(m8_a[:, i, :], sim_a[:, i, :])
                ve.drain()
                ve.tensor_scalar(
                    ge_a[:, i, :], sim_a[:, i, :], m8_a[:, i, 3:4], None,
                    op0=ALU.is_ge,
                )
                ve.wait_ge(s_exp, i + 1)
                ve.drain()
                ve.tensor_tensor(em_a[:, i, :], e_a[:, i, :], ge_a[:, i, :], ALU.mult)
                ve.drain()
                ve.tensor_reduce(
                    den_a[:, i:i + 1], em_a[:, i, :],
                    axis=mybir.AxisListType.X, op=ALU.add,
                )
                ve.drain()
                ve.reciprocal(rcp_a[:, i:i + 1], den_a[:, i:i + 1])
                ve.drain()
                ve.tensor_scalar(
                    co_a[:, i, :], em_a[:, i, :], rcp_a[:, i:i + 1], None,
                    op0=ALU.mult,
                )
                ve.drain()
                # zw slot WAR vs transp(i-2): covered — dve_tail(i-2) waited
                # s_d >= 4(i-2)+4, i.e. mm2(i-2) done, which on PE follows
                # transp(i-1).
                ve.tensor_tensor(
                    zw_sb[:, i % 2],
                    zs_ps[i % 2][:, 0:ER].rearrange("p (e r) -> p e r", r=R),
                    co_a[:, i, :, None].to_broadcast([128, E, R]),
                    ALU.mult,
                ).then_inc(s_zw, 1)
                if i >= 1:
                    dve_tail(i - 1)
            dve_tail(NT - 1)

    nc.finalize()
    return nc


def _host_prep(x, prototypes, A_stack, B_stack, scaling):
    tok = np.ascontiguousarray(x.reshape(T, F))

    Acat = A_stack.reshape(ER, F)
    Wh = np.concatenate([Acat.T, prototypes.T], axis=1)
    Wh = np.ascontiguousarray(
        Wh.reshape(FCH, 128, WCOLS).transpose(1, 0, 2)
    ).astype(np.float16)

    Bcat = (B_stack.transpose(0, 2, 1).reshape(ER, O) * float(scaling))
    Bch = Bcat.reshape(ERCH, 128, O).astype(np.float16)

    identh = np.eye(128, dtype=np.float16)

    in_maps = []
    for core in range(NCORES):
        shard = tok[core * TPC:(core + 1) * TPC]
        xTh = (
            shard.T.reshape(NG, XG, 128, TPC)
            .transpose(0, 2, 1, 3)
            .astype(np.float16)
        )
        in_maps.append({
            "xT": np.ascontiguousarray(xTh),
            "W": Wh,
            "Bc": Bch,
            "ident": identh,
        })
    return in_maps


def _setup_axon_tracing():
    import sys
    import types

    import concourse.bass_utils as bu

    bu.upload_artifacts = lambda tmpdir: "local://" + tmpdir
    try:
        from antenv.axon_hooks import get_axon_ntff_profile_hook  # noqa: F401
        return
    except ImportError:
        pass
    import antenv
    from trn_agent_boot.trn_boot import _ntff_profile_via_ctypes

    mod = types.ModuleType("antenv.axon_hooks")
    state = {"hook": _ntff_profile_via_ctypes("/opt/axon/libaxon_pjrt.so")}
    mod.set_axon_ntff_profile_hook = lambda h: state.__setitem__("hook", h)
    mod.get_axon_ntff_profile_hook = lambda: state["hook"]
    antenv.axon_hooks = mod
    sys.modules["antenv.axon_hooks"] = mod


def kernel(x, prototypes, A_stack, B_stack, scaling, top_k):
    global LAST_RESULTS
    assert int(top_k) == TOPK, f"kernel hardcodes top_k={TOPK}, got {top_k}"
    assert x.shape == (B, S, F)

    if "nc" not in _CACHE:
        _CACHE["nc"] = build_nc()
    nc = _CACHE["nc"]

    in_maps = _host_prep(
        np.asarray(x, dtype=np.float32),
        np.asarray(prototypes, dtype=np.float32),
        np.asarray(A_stack, dtype=np.float32),
        np.asarray(B_stack, dtype=np.float32),
        np.asarray(scaling, dtype=np.float32),
    )

    trace = os.environ.get("KERNEL_TRACE", "0") == "1"
    if trace:
        try:
            _setup_axon_tracing()
        except Exception as e:
            print(f"tracing setup failed ({e}); running without trace")
            trace = False
    res = run_bass_kernel_spmd(nc, in_maps, core_ids=list(range(NCORES)), trace=trace)
    LAST_RESULTS = res

    outs = [
        res.results[i]["out"].transpose(0, 2, 1, 3).reshape(TPC, O)
        for i in range(NCORES)
    ]
    full = np.concatenate(outs, axis=0).astype(np.float32)
    return full.reshape(B, S, O)
